# revision 1
# baseline (speedup 1.0000x reference)
"""Bass/Trainium2 kernel for LightweightHypersphericalAttention.

Sharding: 8 cores = (batch b in 0..3) x (query half in 0..1).
Each core gets x_sh [1024, 512] (its query rows), ctx [2048, 512] (full
context for its batch), the weights, and radius; computes its [1024, 512]
slice of the final output. No collectives; host concatenates slices.

Math per (core, head h):
  qT_raw[d, n] = sum_c WT[c, h*256+d] * xT[c, n]          (PE, d-major)
  kT_raw[d, m] = sum_c WT[c, h*256+d] * cT[c, m]
  f_q[n] = r_h / max(||q[:,n]||, eps); f_k[m] likewise * SCALE
  qT_hat = qT_raw * f_q[n]  (f_q broadcast across partitions via PE trick)
  S^T[m, n] = sum_d kT_raw[d, m] * qT_hat[d, n]            (PSUM)
  P^T = exp(S^T * f_k[m])   (ACT, per-partition scale; logits bounded by
                             sqrt(128) so no max subtraction needed)
  out[n, dv (+denom)] += P^T_chunk^T @ [v | 1]             (PE accumulate)
  out_h = out[:, :128] / out[:, 128]                       (per-partition)
  outcatT[ci, n] = out_h^T  (PE transpose) ; final = outcatT^T @ WpT
"""

import numpy as np

P = 128
B, N, M, C, H = 4, 2048, 2048, 512, 4
D_V = 128
D_QK = 256
SCALE = float(D_V) ** -0.5
EPS = 1e-12
N_CORE = 1024          # query rows per core
NN = N_CORE // P       # 8 query chunks
MM = M // P            # 16 key chunks
CCH = C // P           # 4 channel chunks
NT = N_CORE // 512     # 2 query 512-tiles
MT = M // 512          # 4 key 512-tiles

_NC_CACHE = {}


def _build(mm_bf16: bool):
    import concourse.bass as bass
    import concourse.mybir as mybir
    import concourse.tile as tile
    from concourse import bacc
    from concourse.masks import make_identity

    f32 = mybir.dt.float32
    mdt = mybir.dt.bfloat16 if mm_bf16 else f32

    nc = bacc.Bacc(None, target_bir_lowering=False, debug=False)
    x_t = nc.dram_tensor("x_sh", [N_CORE, C], f32, kind="ExternalInput")
    c_t = nc.dram_tensor("ctx", [M, C], f32, kind="ExternalInput")
    wq_t = nc.dram_tensor("w_qkv", [2 * C, C], f32, kind="ExternalInput")
    wp_t = nc.dram_tensor("w_proj", [C, C], f32, kind="ExternalInput")
    rad_t = nc.dram_tensor("radius", [H], f32, kind="ExternalInput")
    out_t = nc.dram_tensor("out_sh", [N_CORE, C], f32, kind="ExternalOutput")

    from contextlib import ExitStack
    with tile.TileContext(nc) as tc, ExitStack() as es:
        const = es.enter_context(tc.tile_pool(name="const", bufs=1))
        wpool = es.enter_context(tc.tile_pool(name="wpool", bufs=1))
        big = es.enter_context(tc.tile_pool(name="big", bufs=1))
        ld = es.enter_context(tc.tile_pool(name="ld", bufs=3))
        hp = es.enter_context(tc.tile_pool(name="hp", bufs=1))
        sq = es.enter_context(tc.tile_pool(name="sq", bufs=2))
        ptp = es.enter_context(tc.tile_pool(name="ptp", bufs=3))
        outp = es.enter_context(tc.tile_pool(name="outp", bufs=3))
        fp = es.enter_context(tc.tile_pool(name="fp", bufs=2))
        ps_s = es.enter_context(tc.tile_pool(name="ps_s", bufs=2, space="PSUM"))
        ps_av = es.enter_context(tc.tile_pool(name="ps_av", bufs=2, space="PSUM"))
        ps_m = es.enter_context(tc.tile_pool(name="ps_m", bufs=2, space="PSUM"))
        drp = es.enter_context(tc.tile_pool(name="drp", bufs=2, space="DRAM"))

        identity = const.tile([P, P], f32)
        make_identity(nc, identity)
        ones_col_m = const.tile([P, 1], mdt)
        nc.vector.memset(ones_col_m, 1.0)
        # D_all[:, j, i] = 1 if i == j else 0 — selector columns for placing
        # ones-matmul column sums into psum row j.
        D_all = const.tile([P, MT, MT], f32)
        nc.vector.memset(D_all, 0.0)
        for j in range(MT):
            nc.vector.memset(D_all[:, j, j:j + 1], 1.0)
        rad_b = const.tile([P, H], f32)
        rad_ap = rad_t[:]
        rad_bcast_ap = bass.AP(tensor=rad_ap.tensor, offset=rad_ap.offset,
                               ap=[[0, P], rad_ap.ap[0]])
        nc.sync.dma_start(out=rad_b, in_=rad_bcast_ap)
        rad_s = const.tile([P, H], f32)
        nc.scalar.mul(out=rad_s, in_=rad_b, mul=SCALE)

        # ---- transpose weights: WT[c, cc, o], WpT[ci, cc, co] ----
        WT = wpool.tile([P, CCH, 2 * C], mdt)
        wq_nat = wq_t[:].rearrange("(oo p) c -> p oo c", p=P)
        for oo in range(2 * C // P):
            w_nat = ld.tile([P, C], f32, tag="nat")
            nc.sync.dma_start(out=w_nat, in_=wq_nat[:, oo, :])
            for cc in range(CCH):
                pst = ps_m.tile([P, 512], f32, tag="m")
                nc.tensor.transpose(pst[:, :P], w_nat[:, cc * P:(cc + 1) * P],
                                    identity)
                nc.any.tensor_copy(out=WT[:, cc, oo * P:(oo + 1) * P],
                                   in_=pst[:, :P])
        WpT = wpool.tile([P, CCH, C], mdt)
        wp_nat = wp_t[:].rearrange("(oo p) c -> p oo c", p=P)
        for oo in range(CCH):
            w_nat = ld.tile([P, C], f32, tag="nat")
            nc.sync.dma_start(out=w_nat, in_=wp_nat[:, oo, :])
            for cc in range(CCH):
                pst = ps_m.tile([P, 512], f32, tag="m")
                nc.tensor.transpose(pst[:, :P], w_nat[:, cc * P:(cc + 1) * P],
                                    identity)
                nc.any.tensor_copy(out=WpT[:, cc, oo * P:(oo + 1) * P],
                                   in_=pst[:, :P])

        # ---- transpose activations: xT[c, cc, n], cT[c, cc, m] ----
        xT = big.tile([P, CCH, N_CORE], mdt, tag="xT")
        x_nat_ap = x_t[:].rearrange("(nn p) c -> p nn c", p=P)
        for nn in range(NN):
            a_nat = ld.tile([P, C], f32, tag="nat")
            nc.sync.dma_start(out=a_nat, in_=x_nat_ap[:, nn, :])
            for cc in range(CCH):
                pst = ps_m.tile([P, 512], f32, tag="m")
                nc.tensor.transpose(pst[:, :P], a_nat[:, cc * P:(cc + 1) * P],
                                    identity)
                nc.any.tensor_copy(out=xT[:, cc, nn * P:(nn + 1) * P],
                                   in_=pst[:, :P])
        cT = big.tile([P, CCH, M], mdt, tag="cT")
        c_nat_ap = c_t[:].rearrange("(nn p) c -> p nn c", p=P)
        for nn in range(MM):
            a_nat = ld.tile([P, C], f32, tag="nat")
            nc.sync.dma_start(out=a_nat, in_=c_nat_ap[:, nn, :])
            for cc in range(CCH):
                pst = ps_m.tile([P, 512], f32, tag="m")
                nc.tensor.transpose(pst[:, :P], a_nat[:, cc * P:(cc + 1) * P],
                                    identity)
                nc.any.tensor_copy(out=cT[:, cc, nn * P:(nn + 1) * P],
                                   in_=pst[:, :P])

        outcatT = big.tile([P, H, N_CORE], mdt, tag="ocT")

        for h in range(H):
            # ---- project qT_raw [d, 2, n], kT_raw [d, 2, m] ----
            qT = hp.tile([P, 2, N_CORE], mdt, tag="qT")
            for dc in range(2):
                for nt in range(NT):
                    psq = ps_m.tile([P, 512], f32, tag="m")
                    for cc in range(CCH):
                        nc.tensor.matmul(
                            psq,
                            WT[:, cc, h * D_QK + dc * P: h * D_QK + (dc + 1) * P],
                            xT[:, cc, nt * 512:(nt + 1) * 512],
                            start=(cc == 0), stop=(cc == CCH - 1))
                    nc.any.tensor_copy(out=qT[:, dc, nt * 512:(nt + 1) * 512],
                                       in_=psq)
            kT = hp.tile([P, 2, M], mdt, tag="kT")
            for dc in range(2):
                for mt in range(MT):
                    psk = ps_m.tile([P, 512], f32, tag="m")
                    for cc in range(CCH):
                        nc.tensor.matmul(
                            psk,
                            WT[:, cc, h * D_QK + dc * P: h * D_QK + (dc + 1) * P],
                            cT[:, cc, mt * 512:(mt + 1) * 512],
                            start=(cc == 0), stop=(cc == CCH - 1))
                    nc.any.tensor_copy(out=kT[:, dc, mt * 512:(mt + 1) * 512],
                                       in_=psk)

            # ---- sum of squares over d (partition dim) via selector matmul:
            # psum row nt/mt accumulates the column sums of that 512-tile.
            sb_ss_q = fp.tile([P, 512], f32, tag="ssq")
            nc.vector.memset(sb_ss_q, 0.0)
            sb_ss_k = fp.tile([P, 512], f32, tag="ssk")
            nc.vector.memset(sb_ss_k, 0.0)
            pss_q = ps_m.tile([P, 512], f32, tag="m")
            for nt in range(NT):
                sqt = sq.tile([P, 2, 512], f32, tag="sq")
                nc.vector.tensor_mul(sqt, qT[:, :, nt * 512:(nt + 1) * 512],
                                     qT[:, :, nt * 512:(nt + 1) * 512])
                for dc in range(2):
                    nc.tensor.matmul(pss_q[:NT, :], D_all[:, nt, :NT],
                                     sqt[:, dc, :],
                                     start=(nt == 0 and dc == 0),
                                     stop=(nt == NT - 1 and dc == 1))
            nc.any.tensor_copy(out=sb_ss_q[:NT, :], in_=pss_q[:NT, :])
            pss_k = ps_m.tile([P, 512], f32, tag="m")
            for mt in range(MT):
                sqt = sq.tile([P, 2, 512], f32, tag="sq")
                nc.vector.tensor_mul(sqt, kT[:, :, mt * 512:(mt + 1) * 512],
                                     kT[:, :, mt * 512:(mt + 1) * 512])
                for dc in range(2):
                    nc.tensor.matmul(pss_k[:MT, :], D_all[:, mt, :],
                                     sqt[:, dc, :],
                                     start=(mt == 0 and dc == 0),
                                     stop=(mt == MT - 1 and dc == 1))
            nc.any.tensor_copy(out=sb_ss_k[:MT, :], in_=pss_k[:MT, :])

            # chunk-major per-partition layout: col nn = nt*4+b (resp. mt*4+b)
            fq_ss = fp.tile([P, NN], f32, tag="fq")
            fq_v = fq_ss.rearrange("p (nt b) -> p nt b", b=4)
            fk_ss = fp.tile([P, MM], f32, tag="fk")
            fk_v = fk_ss.rearrange("p (mt b) -> p mt b", b=4)
            for b in range(4):
                pst = ps_m.tile([P, 512], f32, tag="m")
                nc.tensor.transpose(pst[:, :P],
                                    sb_ss_q[:, b * P:(b + 1) * P], identity)
                nc.any.tensor_copy(out=fq_v[:, :, b], in_=pst[:, :NT])
                pst2 = ps_m.tile([P, 512], f32, tag="m")
                nc.tensor.transpose(pst2[:, :P],
                                    sb_ss_k[:, b * P:(b + 1) * P], identity)
                nc.any.tensor_copy(out=fk_v[:, :, b], in_=pst2[:, :MT])

            # f = r / max(sqrt(ss), eps)   (fk additionally * SCALE)
            nc.scalar.sqrt(fq_ss, fq_ss)
            nc.vector.tensor_scalar_max(fq_ss, fq_ss, EPS)
            nc.vector.reciprocal(fq_ss, fq_ss)
            nc.vector.tensor_scalar_mul(fq_ss, fq_ss, rad_b[:, h:h + 1])
            nc.scalar.sqrt(fk_ss, fk_ss)
            nc.vector.tensor_scalar_max(fk_ss, fk_ss, EPS)
            nc.vector.reciprocal(fk_ss, fk_ss)
            nc.vector.tensor_scalar_mul(fk_ss, fk_ss, rad_s[:, h:h + 1])

            # ---- broadcast f_q across partitions (DRAM bounce) and apply ----
            scr = drp.tile([N_CORE], f32, tag="scr", name=f"scr_{h}")
            nc.sync.dma_start(out=scr[:].rearrange("(j p) -> p j", p=P),
                              in_=fq_ss)
            fq_b = fp.tile([P, N_CORE], f32, tag="fqb")
            scr_ap = scr[:]
            nc.sync.dma_start(
                out=fq_b,
                in_=bass.AP(tensor=scr_ap.tensor, offset=scr_ap.offset,
                            ap=[[0, P], scr_ap.ap[0]]))
            for nt in range(NT):
                nc.vector.tensor_tensor(
                    qT[:, :, nt * 512:(nt + 1) * 512],
                    qT[:, :, nt * 512:(nt + 1) * 512],
                    fq_b[:, None, nt * 512:(nt + 1) * 512].to_broadcast(
                        (P, 2, 512)),
                    mybir.AluOpType.mult)

            # ---- v tiles [m_p, 16, 128] ----
            v_aug = hp.tile([P, MM, D_V], mdt, tag="vaug")
            v_src = c_t[:, h * D_V:(h + 1) * D_V].rearrange(
                "(j p) dv -> p j dv", p=P)
            nc.gpsimd.dma_start(out=v_aug, in_=v_src)

            # ---- flash loop: out^T[dv, n] accumulation + ones-row denom ----
            for nt in range(NT):
                avo = ps_av.tile([P, 512], f32, tag="av",
                                 name=f"avo_{h}_{nt}")
                dns = ps_av.tile([P, 512], f32, tag="dn",
                                 name=f"dns_{h}_{nt}")
                for j in range(MM):
                    psS = ps_s.tile([P, 512], f32, tag="s")
                    for dc in range(2):
                        nc.tensor.matmul(psS,
                                         kT[:, dc, j * P:(j + 1) * P],
                                         qT[:, dc, nt * 512:(nt + 1) * 512],
                                         start=(dc == 0), stop=(dc == 1))
                    PT = ptp.tile([P, 512], mdt, tag="pt")
                    nc.scalar.activation(PT, psS,
                                         mybir.ActivationFunctionType.Exp,
                                         scale=fk_ss[:, j:j + 1])
                    nc.tensor.matmul(avo, v_aug[:, j, :], PT,
                                     start=(j == 0), stop=(j == MM - 1))
                    nc.tensor.matmul(dns[:1, :], ones_col_m, PT,
                                     start=(j == 0), stop=(j == MM - 1))
                # normalize: outcatT[:, h, nt] = avo * (1/denom) bcast
                drow = outp.tile([P, 512], f32, tag="drow")
                nc.vector.reciprocal(drow[:1, :], dns[:1, :])
                scr2 = drp.tile([512], f32, tag="scr2", name=f"scr2_{h}_{nt}")
                nc.sync.dma_start(
                    out=scr2[:].rearrange("(one n) -> one n", one=1),
                    in_=drow[:1, :])
                db = fp.tile([P, 512], f32, tag="db")
                s2ap = scr2[:]
                nc.sync.dma_start(
                    out=db,
                    in_=bass.AP(tensor=s2ap.tensor, offset=s2ap.offset,
                                ap=[[0, P], s2ap.ap[0]]))
                nc.vector.tensor_tensor(
                    outcatT[:, h, nt * 512:(nt + 1) * 512],
                    avo, db, mybir.AluOpType.mult)

        # ---- output projection ----
        for nn in range(NN):
            pso = ps_m.tile([P, 512], f32, tag="m")
            for cc in range(CCH):
                nc.tensor.matmul(pso, outcatT[:, cc, nn * P:(nn + 1) * P],
                                 WpT[:, cc, :],
                                 start=(cc == 0), stop=(cc == CCH - 1))
            o_sb = outp.tile([P, C], f32, tag="osb")
            nc.any.tensor_copy(out=o_sb, in_=pso)
            nc.sync.dma_start(out=out_t[nn * P:(nn + 1) * P, :], in_=o_sb)

    nc.compile()
    return nc


def _get_nc(mm_bf16: bool):
    if mm_bf16 not in _NC_CACHE:
        _NC_CACHE[mm_bf16] = _build(mm_bf16)
    return _NC_CACHE[mm_bf16]


def kernel(x, context, W_qkv, W_proj, radius, _trace=False, _bf16=True):
    from concourse.bass_utils import run_bass_kernel_spmd

    x = np.ascontiguousarray(np.asarray(x, dtype=np.float32))
    context = np.ascontiguousarray(np.asarray(context, dtype=np.float32))
    W_qkv = np.ascontiguousarray(np.asarray(W_qkv, dtype=np.float32))
    W_proj = np.ascontiguousarray(np.asarray(W_proj, dtype=np.float32))
    radius = np.ascontiguousarray(np.asarray(radius, dtype=np.float32))

    nc = _get_nc(_bf16)
    in_maps = []
    for i in range(8):
        b, half = i // 2, i % 2
        in_maps.append({
            "x_sh": x[b, half * N_CORE:(half + 1) * N_CORE, :],
            "ctx": context[b],
            "w_qkv": W_qkv,
            "w_proj": W_proj,
            "radius": radius,
        })
    res = run_bass_kernel_spmd(nc, in_maps, list(range(8)), trace=_trace)
    out = np.empty((B, N, C), dtype=np.float32)
    for i in range(8):
        b, half = i // 2, i % 2
        out[b, half * N_CORE:(half + 1) * N_CORE, :] = res.results[i]["out_sh"]
    if _trace:
        return out, res
    return out

